# revision 20
# baseline (speedup 1.0000x reference)
"""Multi-head attention forward (B=2, N=2048, C=1024, H=16) on 8 TRN2 NeuronCores.

Tensor-parallel over heads: core c owns heads {2c, 2c+1}. Each core computes
QKV projection for its heads, full attention for its 4 (batch, head)
instances, and a partial output projection against its 128 rows of w_proj.
The host sums the 8 partial projections and adds the bias (row-parallel TP;
the all-reduce is the host-side unshard).

Per-core layouts (all matmul inputs bf16, PSUM accumulation f32):
  xT    [1024, 4096]  x^T, channel-major (replicated)
  wqk   [1024, 256]   [Wq_h0|Wq_h1|Wk_h0|Wk_h1] columns, Wq pre-scaled by D^-0.5
  wv    [1024, 128]   [Wv_h0|Wv_h1]
  wproj [128, 1024]   rows 128c:128c+128 of w_proj
  out   [4096, 1024]  bf16 partial projection output

Attention processes BOTH heads per (batch, q-block): the two S^T matmuls
(K=64 contraction each) are issued back-to-back with tile_position row
tiles so they run CONCURRENTLY in disjoint halves of the PE array. Per
k-tile, one [128, 1024] PSUM tile holds [S^T_h0 | S^T_h1] and a single ACT
exp covers both heads.

V is computed directly in token-major layout (stationary = x^T chunks,
moving = wv, N=128) -- no PE transposes. Slot layouts:
  slot0 (h0): [V_h0 (64) | 1 | zeros(63)]   -> po0 = [O_h0 0:64; l_h0 @64]
  slot1 (h1): [1 | zeros(63) | V_h1 (64)]   -> po1 = [l_h1 @0; O_h1 64:128]
so h1's normalized output writes otp partitions 64:128 DIRECTLY (the DVE
mul reads the broadcast tile at partitions 0:64; cross-offset in1 reads are
supported) -- no SBUF repack DMA. l rides the PV matmul via the ones column.

Startup: critical DMAs (wqk + x^T tokens 0:512) spread over all four
queues; ~40 dummy ident matmuls warm the PE HAM clock-gate during the DMA
wait; a short prefix (K tokens 0:256 at N=256, Q tokens 0:512) gets the
first S-pair and exp stream going ~12us earlier than a full-block prefix.

Scheduling: stage-1 and projection PE work is woven into the attention
loops' ACT-wait slots. Exp->O matmuls are skewed 3 kp iterations; each
q-block's last 6 O-accumulations drain 3-per-kp into the next block's
kp0/kp1 with the normalize chain issued at kp1 so the po PSUM banks free
before the block's own O pops begin at kp3.
"""

import numpy as np
import ml_dtypes

import concourse.bass as bass
import concourse.tile as tile
from concourse import bacc, mybir
from concourse.bass_utils import run_bass_kernel_spmd
from concourse.masks import make_identity

B, N, C = 2, 2048, 1024
H = 16
D = C // H          # 64
SCALE = D ** -0.5
NCORES = 8
T = B * N           # 4096 tokens
KT = C // 128       # 8 k-tiles over the C contraction
NK = N // 128       # 16 key tiles per sequence
QB = 512            # q block width
BF = mybir.dt.bfloat16
F32 = mybir.dt.float32

_NC_CACHE = {}


def build():
    nc = bacc.Bacc("TRN2", target_bir_lowering=False, debug=False,
                   num_devices=NCORES)
    xT = nc.dram_tensor("xT", [C, T], BF, kind="ExternalInput").ap()
    wqk = nc.dram_tensor("wqk", [C, 256], BF, kind="ExternalInput").ap()
    wv = nc.dram_tensor("wv", [C, 128], BF, kind="ExternalInput").ap()
    wproj = nc.dram_tensor("wproj", [128, C], BF, kind="ExternalInput").ap()
    out = nc.dram_tensor("out", [T, C], BF, kind="ExternalOutput").ap()

    with tile.TileContext(nc) as tc:
        with tc.tile_pool(name="const", bufs=1) as const, \
             tc.tile_pool(name="work", bufs=2) as work, \
             tc.tile_pool(name="ps", bufs=2, space="PSUM") as ps:

            xt_sb = const.tile([128, KT, T], BF, tag="xt")
            wqk_sb = const.tile([128, KT, 256], BF, tag="wqk")
            wv_sb = const.tile([128, KT, 128], BF, tag="wv")
            wproj_sb = const.tile([128, C], BF, tag="wproj")
            qk_sb = const.tile([128, 2, T], BF, tag="qk")   # [qchan|kchan, token]
            # v slots per 128-token tile: [h0: V|1|0pad, h1: 0pad|1|V]
            v_sb = const.tile([128, T // 128, 2, 128], BF, tag="v")
            otp_sb = const.tile([128, T], BF, tag="otp")    # normalized O^T packed
            ident = const.tile([128, 128], BF, tag="ident")

            # ---- critical DMAs: wqk (8 chunks) + x^T tokens 0:512 (8
            # chunks) spread as the FIRST 4 descriptors on each of the four
            # queues. Everything later goes on sync/gpsimd.
            def xt_dma(eng, nt, kt):
                eng.dma_start(
                    out=xt_sb[:, kt, nt * 512:(nt + 1) * 512],
                    in_=xT[kt * 128:(kt + 1) * 128,
                           nt * 512:(nt + 1) * 512])

            for kt in range(KT):
                (nc.sync if kt % 2 == 0 else nc.scalar).dma_start(
                    out=wqk_sb[:, kt, :], in_=wqk[kt * 128:(kt + 1) * 128, :])
                if kt % 2 == 0:
                    xt_dma(nc.gpsimd, 0, kt)
                elif kt % 4 == 1:
                    xt_dma(nc.sync, 0, kt)
                elif kt == 3:
                    xt_dma(nc.scalar, 0, kt)
                else:
                    xt_dma(nc.gpsimd, 0, kt)

            # ---- v-slot constants; then HAM-prewarm dummy matmuls (no DMA
            # deps) so the PE clock-gate opens during the DMA wait
            nc.vector.memset(ident[:], 1.0)
            nc.vector.memset(v_sb[:, :, 0, 64:65], 1.0)
            nc.vector.memset(v_sb[:, :, 1, 0:1], 1.0)
            pwarm = ps.tile([128, 512], F32, tag="mm")
            for i in range(40):
                nc.tensor.matmul(pwarm[:, 0:128], ident[:], ident[:],
                                 start=True, stop=True)

            # ---- remaining input DMAs on sync/gpsimd only
            for kt in range(KT):
                xt_dma(nc.sync if kt % 2 == 0 else nc.gpsimd, 1, kt)
                nc.gpsimd.dma_start(out=wv_sb[:, kt, :],
                                    in_=wv[kt * 128:(kt + 1) * 128, :])
            nc.gpsimd.dma_start(out=wproj_sb[:], in_=wproj[:, :])
            for nt in range(2, T // 512):
                for kt in range(KT):
                    xt_dma(nc.sync if kt % 2 == 0 else nc.gpsimd, nt, kt)

            # -- work units -------------------------------------------------
            def emit_vzero(c):
                # zero-pad of v slots for tiles 8c:8c+8 (pads must be
                # initialized before the first O pop reads those slots)
                nc.vector.memset(v_sb[:, 8 * c:8 * (c + 1), 0, 65:128], 0.0)
                nc.vector.memset(v_sb[:, 8 * c:8 * (c + 1), 1, 1:64], 0.0)

            def emit_qk(mt, nt):
                pmm = ps.tile([128, 512], F32, tag="mm")
                for kt in range(KT):
                    nc.tensor.matmul(
                        pmm[:],
                        wqk_sb[:, kt, mt * 128:(mt + 1) * 128],
                        xt_sb[:, kt, nt * 512:(nt + 1) * 512],
                        start=(kt == 0), stop=(kt == KT - 1))
                nc.vector.tensor_copy(
                    qk_sb[:, mt, nt * 512:(nt + 1) * 512], pmm[:])

            def emit_qk_split(mt, nt):
                # 8-chunk GEMM split into two 4-chunk halves emitted in
                # consecutive extras slots (accumulation group + psum tile
                # stay open across the pair)
                st = {}

                def a():
                    pmm = ps.tile([128, 512], F32, tag="mm", name="pmm")
                    st["p"] = pmm
                    for kt in range(KT // 2):
                        nc.tensor.matmul(
                            pmm[:],
                            wqk_sb[:, kt, mt * 128:(mt + 1) * 128],
                            xt_sb[:, kt, nt * 512:(nt + 1) * 512],
                            start=(kt == 0), stop=False)

                def b():
                    pmm = st["p"]
                    for kt in range(KT // 2, KT):
                        nc.tensor.matmul(
                            pmm[:],
                            wqk_sb[:, kt, mt * 128:(mt + 1) * 128],
                            xt_sb[:, kt, nt * 512:(nt + 1) * 512],
                            start=False, stop=(kt == KT - 1))
                    nc.vector.tensor_copy(
                        qk_sb[:, mt, nt * 512:(nt + 1) * 512], pmm[:])

                return a, b

            def emit_k256(j):
                # K chunk for tokens j*256:(j+1)*256 (k-tiles 2j, 2j+1);
                # one N=256 GEMM + one CAST, fine-grained for early blocks
                pmm = ps.tile([128, 512], F32, tag="mm")
                for kt in range(KT):
                    nc.tensor.matmul(
                        pmm[:, 0:256],
                        wqk_sb[:, kt, 128:256],
                        xt_sb[:, kt, j * 256:(j + 1) * 256],
                        start=(kt == 0), stop=(kt == KT - 1))
                nc.vector.tensor_copy(
                    qk_sb[:, 1, j * 256:(j + 1) * 256], pmm[:, 0:256])

            def emit_v(t):
                # token-major V for 128-token tile t, BOTH heads:
                # out[tok, vchan] = sum_k xT[k, tok-tile]^T @ wv[k, :]
                pv = ps.tile([128, 512], F32, tag="mm")
                for kt in range(KT):
                    nc.tensor.matmul(
                        pv[:, 0:128],
                        xt_sb[:, kt, t * 128:(t + 1) * 128],
                        wv_sb[:, kt, :],
                        start=(kt == 0), stop=(kt == KT - 1))
                nc.vector.tensor_copy(v_sb[:, t, 0, 0:64], pv[:, 0:64])
                nc.vector.tensor_copy(v_sb[:, t, 1, 64:128], pv[:, 64:128])

            def emit_proj(g, evac=None, dma_eng=None):
                # out_tile = O^T_packed.T @ wproj (K=128, both heads)
                ob = work.tile([128, C], BF, tag="outstage", bufs=4)
                for ntile in range(2):
                    pmm = ps.tile([128, 512], F32, tag="mm")
                    nc.tensor.matmul(
                        pmm[:],
                        otp_sb[:, g * 128:(g + 1) * 128],
                        wproj_sb[:, ntile * 512:(ntile + 1) * 512],
                        start=True, stop=True)
                    if evac == "scalar":
                        nc.scalar.copy(
                            ob[:, ntile * 512:(ntile + 1) * 512], pmm[:])
                    elif evac == "mixed" and ntile == 1:
                        nc.scalar.copy(ob[:, 512:1024], pmm[:])
                    else:
                        nc.vector.tensor_copy(
                            ob[:, ntile * 512:(ntile + 1) * 512], pmm[:])
                (dma_eng or nc.sync).dma_start(
                    out=out[g * 128:(g + 1) * 128, :], in_=ob[:])

            # One attention q-block, BOTH heads.
            def emit_s2_pair(b, qb, extras=None, post=None, carry=None,
                             last=False):
                q0 = b * N + qb * QB
                po0 = ps.tile([128, 512], F32, tag="o0", bufs=1)
                po1 = ps.tile([128, 512], F32, tag="o1", bufs=1)
                from collections import deque as _dq
                pending = _dq()

                def pop_o():
                    es, kt = pending.popleft()
                    vt = b * NK + kt
                    nc.tensor.matmul(
                        po0[:], v_sb[:, vt, 0, :], es[:, 0:512],
                        start=(kt == 0), stop=(kt == NK - 1))
                    nc.tensor.matmul(
                        po1[:], v_sb[:, vt, 1, :], es[:, 512:1024],
                        start=(kt == 0), stop=(kt == NK - 1))

                def normalize(heads=(0, 1), chunks=1):
                    # h0: l at po0 row 64, O at rows 0:64 -> otp[0:64]
                    # h1: l at po1 row 0, O at rows 64:128 -> otp[64:128]
                    cw = QB // chunks
                    for h in heads:
                        po = po0 if h == 0 else po1
                        lrow = work.tile([1, 512], F32, tag="lrow")
                        nc.vector.tensor_copy(
                            lrow[:], po[64:65, :] if h == 0 else po[0:1, :])
                        nc.vector.reciprocal_approx_fast(lrow[:], lrow[:])
                        lb = work.tile([64, 512], F32, tag="lb")
                        nc.gpsimd.partition_broadcast(lb[:], lrow[:])
                        for c in range(chunks):
                            s = c * cw
                            if h == 0:
                                nc.vector.tensor_mul(
                                    otp_sb[0:64, q0 + s:q0 + s + cw],
                                    po[0:64, s:s + cw], lb[:, s:s + cw])
                            else:
                                nc.vector.tensor_mul(
                                    otp_sb[64:128, q0 + s:q0 + s + cw],
                                    po[64:128, s:s + cw], lb[:, s:s + cw])

                def s_exp(kp, j):
                    kt = kp * 2 + j
                    k0 = b * N + kt * 128
                    pst = ps.tile([128, 1024], F32, tag="s")
                    nc.tensor.matmul(
                        pst[:, 0:512],
                        qk_sb[0:64, 1, k0:k0 + 128],
                        qk_sb[0:64, 0, q0:q0 + QB],
                        start=True, stop=True)
                    nc.tensor.matmul(
                        pst[:, 512:1024],
                        qk_sb[64:128, 1, k0:k0 + 128],
                        qk_sb[64:128, 0, q0:q0 + QB],
                        start=True, stop=True)
                    es = work.tile([128, 1024], BF, tag="es", bufs=13)
                    nc.scalar.activation(
                        es[:], pst[:], mybir.ActivationFunctionType.Exp)
                    pending.append((es, kt))

                for kp in range(NK // 2):
                    if carry is not None:
                        carry[0]()
                        carry[0]()
                        if carry[1]():
                            carry = None
                    else:
                        if len(pending) >= 6:
                            pop_o()
                            pop_o()
                    if extras:
                        for u in extras.get(kp, ()):
                            u()
                    s_exp(kp, 0)
                    s_exp(kp, 1)
                    if post:
                        for u in post.get(kp, ()):
                            u()
                if last:
                    # drain head-by-head: h0's normalize chain overlaps h1's
                    # remaining O matmuls on the PE; normalize in 256-col
                    # halves so the first tail projections start early
                    rem = list(pending)
                    pending.clear()
                    for es, kt in rem:
                        nc.tensor.matmul(
                            po0[:], v_sb[:, b * NK + kt, 0, :], es[:, 0:512],
                            start=(kt == 0), stop=(kt == NK - 1))
                    normalize(heads=(0,))
                    for es, kt in rem:
                        nc.tensor.matmul(
                            po1[:], v_sb[:, b * NK + kt, 1, :],
                            es[:, 512:1024],
                            start=(kt == 0), stop=(kt == NK - 1))
                    normalize(heads=(1,))
                    return None

                def carry_pop():
                    if len(pending) > 2:
                        pop_o()

                def carry_fin():
                    if len(pending) > 2:
                        return False
                    rem = list(pending)
                    pending.clear()
                    for es, kt in rem:
                        nc.tensor.matmul(
                            po0[:], v_sb[:, b * NK + kt, 0, :], es[:, 0:512],
                            start=(kt == 0), stop=(kt == NK - 1))
                    normalize(heads=(0,))
                    for es, kt in rem:
                        nc.tensor.matmul(
                            po1[:], v_sb[:, b * NK + kt, 1, :],
                            es[:, 512:1024],
                            start=(kt == 0), stop=(kt == NK - 1))
                    normalize(heads=(1,))
                    return True

                return (carry_pop, carry_fin)

            # -- schedule ---------------------------------------------------
            def U(f, *a):
                return lambda: f(*a)

            # minimal prefix for (b0, qb0): K tokens 0:256 (N=256) and the
            # full Q block 0:512, K/Q interleaved per k-chunk
            pK = ps.tile([128, 512], F32, tag="mm")
            pQ = ps.tile([128, 512], F32, tag="mm")
            for kt in range(KT):
                nc.tensor.matmul(pK[:, 0:256], wqk_sb[:, kt, 128:256],
                                 xt_sb[:, kt, 0:256],
                                 start=(kt == 0), stop=(kt == KT - 1))
                nc.tensor.matmul(pQ[:], wqk_sb[:, kt, 0:128],
                                 xt_sb[:, kt, 0:512],
                                 start=(kt == 0), stop=(kt == KT - 1))
            nc.vector.tensor_copy(qk_sb[:, 1, 0:256], pK[:, 0:256])
            nc.vector.tensor_copy(qk_sb[:, 0, 0:512], pQ[:])

            # (0,0): k256(j) covers b0 K tokens 256j:256j+256 (needed by
            # kp(j-1)); v_t needed by the pop at kp(t//2+3), so emitted by
            # kp(t//2+2); Q(0,1) by next block. kp0/kp1 extras run AFTER the
            # s_exps so the first exps start as early as possible.
            cy = emit_s2_pair(0, 0, post={
                0: [U(emit_k256, 1), U(emit_k256, 2), U(emit_vzero, 0)],
                1: [U(emit_k256, 3), U(emit_k256, 4), U(emit_v, 0)],
            }, extras={
                2: [U(emit_v, 1), U(emit_v, 2), U(emit_v, 3),
                    U(emit_vzero, 1)],
                3: [U(emit_k256, 5), U(emit_v, 4), U(emit_v, 5)],
                4: [U(emit_k256, 6), U(emit_v, 6), U(emit_v, 7),
                    U(emit_vzero, 2)],
                5: [U(emit_k256, 7), U(emit_v, 8), U(emit_v, 9),
                    U(emit_vzero, 3)],
                6: [U(emit_qk, 0, 1), U(emit_v, 10), U(emit_v, 11)],
                7: [U(emit_v, 12), U(emit_v, 13)],
            })
            qk14 = emit_qk_split(1, 4)
            qk02 = emit_qk_split(0, 2)
            qk03 = emit_qk_split(0, 3)
            cy = emit_s2_pair(0, 1, carry=cy, post={
                0: [U(emit_v, 14), U(emit_v, 15)],
            }, extras={
                2: [U(emit_v, 16), U(emit_v, 17)],
                3: [qk02[0]],
                4: [qk02[1], U(emit_v, 18)],
                5: [qk14[0]],
                6: [qk14[1], U(emit_v, 19)],
                7: [qk03[0], U(emit_v, 20)],
            })
            qk15 = emit_qk_split(1, 5)
            qk04 = emit_qk_split(0, 4)
            qk05 = emit_qk_split(0, 5)
            cy = emit_s2_pair(0, 2, carry=cy, extras={
                2: [qk03[1], U(emit_v, 21)],
                3: [qk15[0]],
                4: [qk15[1], U(emit_v, 22)],
                5: [qk04[0]],
                6: [qk04[1], U(emit_v, 23)],
                7: [qk05[0], U(emit_v, 24)],
            })
            qk16 = emit_qk_split(1, 6)
            qk06 = emit_qk_split(0, 6)
            cy = emit_s2_pair(0, 3, carry=cy, extras={
                2: [qk05[1], U(emit_v, 25)],
                3: [qk16[0]],
                4: [qk16[1], U(emit_v, 26)],
                5: [qk06[0]],
                6: [qk06[1], U(emit_proj, 0)],
                7: [U(emit_proj, 1), U(emit_v, 27)],
            })
            qk17 = emit_qk_split(1, 7)
            qk07 = emit_qk_split(0, 7)
            cy = emit_s2_pair(1, 0, carry=cy, extras={
                2: [U(emit_v, 28), U(emit_v, 29)],
                3: [qk17[0]],
                4: [qk17[1], U(emit_v, 30)],
                5: [qk07[0]],
                6: [qk07[1], U(emit_v, 31)],
                7: [U(emit_proj, 2, "mixed"), U(emit_proj, 3, "mixed")],
            })
            cy = emit_s2_pair(1, 1, carry=cy, extras={
                2: [U(emit_proj, 4)],
                3: [U(emit_proj, 5), U(emit_proj, 6)],
                4: [U(emit_proj, 7), U(emit_proj, 8)],
                5: [U(emit_proj, 9), U(emit_proj, 10)],
                6: [U(emit_proj, 11), U(emit_proj, 12)],
                7: [U(emit_proj, 13, "mixed")],
            })
            cy = emit_s2_pair(1, 2, carry=cy, extras={
                2: [U(emit_proj, 14)],
                3: [U(emit_proj, 15), U(emit_proj, 16)],
                4: [U(emit_proj, 17), U(emit_proj, 18)],
                5: [U(emit_proj, 19), U(emit_proj, 20)],
                6: [U(emit_proj, 21), U(emit_proj, 22)],
                7: [U(emit_proj, 23, "mixed")],
            })
            emit_s2_pair(1, 3, carry=cy, last=True, extras={
                2: [U(emit_proj, 24)],
                3: [U(emit_proj, 25)],
                4: [U(emit_proj, 26)],
                5: [U(emit_proj, 27)],
            })
            # tail: ~10 dummy matmuls keep the PE clock-gate open through
            # the final normalize chain; last 4 projections evacuate with one
            # copy on scalar (idle now) and one on vector, out-DMAs spread
            pw2 = ps.tile([128, 512], F32, tag="mm")
            for i in range(28):
                nc.tensor.matmul(pw2[:, 0:128], ident[:], ident[:],
                                 start=True, stop=True)

            def tail_proj(g, dma_a, dma_b):
                ob = work.tile([128, C], BF, tag="outstage", bufs=4)
                for ntile in range(2):
                    pmm = ps.tile([128, 512], F32, tag="mm")
                    nc.tensor.matmul(
                        pmm[:],
                        otp_sb[:, g * 128:(g + 1) * 128],
                        wproj_sb[:, ntile * 512:(ntile + 1) * 512],
                        start=True, stop=True)
                    if ntile == 0:
                        nc.vector.tensor_copy(ob[:, 0:512], pmm[:])
                        dma_a.dma_start(
                            out=out[g * 128:(g + 1) * 128, 0:512],
                            in_=ob[:, 0:512])
                    else:
                        nc.scalar.copy(ob[:, 512:1024], pmm[:])
                        dma_b.dma_start(
                            out=out[g * 128:(g + 1) * 128, 512:1024],
                            in_=ob[:, 512:1024])

            tail_proj(28, nc.sync, nc.scalar)
            tail_proj(29, nc.gpsimd, nc.scalar)
            tail_proj(30, nc.sync, nc.scalar)
            tail_proj(31, nc.gpsimd, nc.sync)
    nc.compile()
    return nc


def make_in_maps(x, w_qkv, w_proj):
    bf = ml_dtypes.bfloat16
    x2 = x.reshape(T, C)
    xT_np = np.ascontiguousarray(x2.T).astype(bf)
    in_maps = []
    for c in range(NCORES):
        s = c * 128
        wq = w_qkv[:, s:s + 128] * SCALE
        wk = w_qkv[:, C + s:C + s + 128]
        wqk_np = np.ascontiguousarray(
            np.concatenate([wq, wk], axis=1)).astype(bf)
        wv_np = np.ascontiguousarray(
            w_qkv[:, 2 * C + s:2 * C + s + 128]).astype(bf)
        wproj_np = np.ascontiguousarray(w_proj[s:s + 128, :]).astype(bf)
        in_maps.append({"xT": xT_np, "wqk": wqk_np, "wv": wv_np,
                        "wproj": wproj_np})
    return in_maps


def kernel(x, w_qkv, w_proj, b_proj):
    x = np.asarray(x, dtype=np.float32)
    w_qkv = np.asarray(w_qkv, dtype=np.float32)
    w_proj = np.asarray(w_proj, dtype=np.float32)
    b_proj = np.asarray(b_proj, dtype=np.float32)

    if "nc" not in _NC_CACHE:
        _NC_CACHE["nc"] = build()
    nc = _NC_CACHE["nc"]

    in_maps = make_in_maps(x, w_qkv, w_proj)
    res = run_bass_kernel_spmd(nc, in_maps, list(range(NCORES)))
    acc = np.zeros((T, C), dtype=np.float32)
    for r in res.results:
        acc += np.asarray(r["out"], dtype=np.float32)
    acc += b_proj[None, :]
    return acc.reshape(B, N, C)


# revision 21
# speedup vs baseline: 1.0220x; 1.0220x over previous
"""Multi-head attention forward (B=2, N=2048, C=1024, H=16) on 8 TRN2 NeuronCores.

Tensor-parallel over heads: core c owns heads {2c, 2c+1}. Each core computes
QKV projection for its heads, full attention for its 4 (batch, head)
instances, and a partial output projection against its 128 rows of w_proj.
The host sums the 8 partial projections and adds the bias (row-parallel TP;
the all-reduce is the host-side unshard).

Per-core layouts (all matmul inputs bf16, PSUM accumulation f32):
  xT    [1024, 4096]  x^T, channel-major (replicated)
  wqk   [1024, 256]   [Wq_h0|Wq_h1|Wk_h0|Wk_h1] columns, Wq pre-scaled by D^-0.5
  wv    [1024, 128]   [Wv_h0|Wv_h1]
  wproj [128, 1024]   rows 128c:128c+128 of w_proj
  out   [4096, 1024]  bf16 partial projection output

Attention processes BOTH heads per (batch, q-block): the two S^T matmuls
(K=64 contraction each) are issued back-to-back with tile_position row
tiles so they run CONCURRENTLY in disjoint halves of the PE array. Per
k-tile, one [128, 1024] PSUM tile holds [S^T_h0 | S^T_h1] and a single ACT
exp covers both heads.

V is computed directly in token-major layout (stationary = x^T chunks,
moving = wv, N=128) -- no PE transposes. Slot layouts:
  slot0 (h0): [V_h0 (64) | 1 | zeros(63)]   -> po0 = [O_h0 0:64; l_h0 @64]
  slot1 (h1): [1 | zeros(63) | V_h1 (64)]   -> po1 = [l_h1 @0; O_h1 64:128]
so h1's normalized output writes otp partitions 64:128 DIRECTLY (the DVE
mul reads the broadcast tile at partitions 0:64; cross-offset in1 reads are
supported) -- no SBUF repack DMA. l rides the PV matmul via the ones column.

Startup: critical DMAs (wqk + x^T tokens 0:512) spread over all four
queues; ~40 dummy ident matmuls warm the PE HAM clock-gate during the DMA
wait; a short prefix (K tokens 0:256 at N=256, Q tokens 0:512) gets the
first S-pair and exp stream going ~12us earlier than a full-block prefix.

Scheduling: stage-1 and projection PE work is woven into the attention
loops' ACT-wait slots. Exp->O matmuls are skewed 3 kp iterations; each
q-block's last 6 O-accumulations drain 3-per-kp into the next block's
kp0/kp1 with the normalize chain issued at kp1 so the po PSUM banks free
before the block's own O pops begin at kp3.
"""

import numpy as np
import ml_dtypes

import concourse.bass as bass
import concourse.tile as tile
from concourse import bacc, mybir
from concourse.bass_utils import run_bass_kernel_spmd
from concourse.masks import make_identity

B, N, C = 2, 2048, 1024
H = 16
D = C // H          # 64
SCALE = D ** -0.5
NCORES = 8
T = B * N           # 4096 tokens
KT = C // 128       # 8 k-tiles over the C contraction
NK = N // 128       # 16 key tiles per sequence
QB = 512            # q block width
BF = mybir.dt.bfloat16
F32 = mybir.dt.float32

_NC_CACHE = {}


def build():
    nc = bacc.Bacc("TRN2", target_bir_lowering=False, debug=False,
                   num_devices=NCORES)
    xT = nc.dram_tensor("xT", [C, T], BF, kind="ExternalInput").ap()
    wqk = nc.dram_tensor("wqk", [C, 256], BF, kind="ExternalInput").ap()
    wv = nc.dram_tensor("wv", [C, 128], BF, kind="ExternalInput").ap()
    wproj = nc.dram_tensor("wproj", [128, C], BF, kind="ExternalInput").ap()
    out = nc.dram_tensor("out", [T, C], BF, kind="ExternalOutput").ap()

    with tile.TileContext(nc) as tc:
        with tc.tile_pool(name="const", bufs=1) as const, \
             tc.tile_pool(name="work", bufs=2) as work, \
             tc.tile_pool(name="ps", bufs=2, space="PSUM") as ps:

            xt_sb = const.tile([128, KT, T], BF, tag="xt")
            wqk_sb = const.tile([128, KT, 256], BF, tag="wqk")
            wv_sb = const.tile([128, KT, 128], BF, tag="wv")
            wproj_sb = const.tile([128, C], BF, tag="wproj")
            qk_sb = const.tile([128, 2, T], BF, tag="qk")   # [qchan|kchan, token]
            # v slots per 128-token tile: [h0: V|1|0pad, h1: 0pad|1|V]
            v_sb = const.tile([128, T // 128, 2, 128], BF, tag="v")
            otp_sb = const.tile([128, T], BF, tag="otp")    # normalized O^T packed
            ident = const.tile([128, 128], BF, tag="ident")

            # ---- critical DMAs: wqk (8 chunks) + x^T tokens 0:512 (8
            # chunks) spread as the FIRST 4 descriptors on each of the four
            # queues. Everything later goes on sync/gpsimd.
            def xt_dma(eng, nt, kt):
                eng.dma_start(
                    out=xt_sb[:, kt, nt * 512:(nt + 1) * 512],
                    in_=xT[kt * 128:(kt + 1) * 128,
                           nt * 512:(nt + 1) * 512])

            for kt in range(KT):
                (nc.sync if kt % 2 == 0 else nc.scalar).dma_start(
                    out=wqk_sb[:, kt, :], in_=wqk[kt * 128:(kt + 1) * 128, :])
                if kt % 2 == 0:
                    xt_dma(nc.gpsimd, 0, kt)
                elif kt % 4 == 1:
                    xt_dma(nc.sync, 0, kt)
                elif kt == 3:
                    xt_dma(nc.scalar, 0, kt)
                else:
                    xt_dma(nc.gpsimd, 0, kt)

            # ---- v-slot constants; then HAM-prewarm dummy matmuls (no DMA
            # deps) so the PE clock-gate opens during the DMA wait
            nc.vector.memset(ident[:], 1.0)
            nc.vector.memset(v_sb[:, :, 0, 64:65], 1.0)
            nc.vector.memset(v_sb[:, :, 1, 0:1], 1.0)
            pwarm = ps.tile([128, 512], F32, tag="mm")
            for i in range(40):
                nc.tensor.matmul(pwarm[:, 0:128], ident[:], ident[:],
                                 start=True, stop=True)

            # ---- remaining input DMAs on sync/gpsimd only
            for kt in range(KT):
                xt_dma(nc.sync if kt % 2 == 0 else nc.gpsimd, 1, kt)
                nc.gpsimd.dma_start(out=wv_sb[:, kt, :],
                                    in_=wv[kt * 128:(kt + 1) * 128, :])
            nc.gpsimd.dma_start(out=wproj_sb[:], in_=wproj[:, :])
            for nt in range(2, T // 512):
                for kt in range(KT):
                    xt_dma(nc.sync if kt % 2 == 0 else nc.gpsimd, nt, kt)

            # -- work units -------------------------------------------------
            def emit_vzero(c):
                # zero-pad of v slots for tiles 8c:8c+8 (pads must be
                # initialized before the first O pop reads those slots)
                nc.vector.memset(v_sb[:, 8 * c:8 * (c + 1), 0, 65:128], 0.0)
                nc.vector.memset(v_sb[:, 8 * c:8 * (c + 1), 1, 1:64], 0.0)

            def emit_qk(mt, nt):
                pmm = ps.tile([128, 512], F32, tag="mm")
                for kt in range(KT):
                    nc.tensor.matmul(
                        pmm[:],
                        wqk_sb[:, kt, mt * 128:(mt + 1) * 128],
                        xt_sb[:, kt, nt * 512:(nt + 1) * 512],
                        start=(kt == 0), stop=(kt == KT - 1))
                nc.vector.tensor_copy(
                    qk_sb[:, mt, nt * 512:(nt + 1) * 512], pmm[:])

            def emit_qk_split(mt, nt):
                # 8-chunk GEMM split into two 4-chunk halves emitted in
                # consecutive extras slots (accumulation group + psum tile
                # stay open across the pair)
                st = {}

                def a():
                    pmm = ps.tile([128, 512], F32, tag="mm", name="pmm")
                    st["p"] = pmm
                    for kt in range(KT // 2):
                        nc.tensor.matmul(
                            pmm[:],
                            wqk_sb[:, kt, mt * 128:(mt + 1) * 128],
                            xt_sb[:, kt, nt * 512:(nt + 1) * 512],
                            start=(kt == 0), stop=False)

                def b():
                    pmm = st["p"]
                    for kt in range(KT // 2, KT):
                        nc.tensor.matmul(
                            pmm[:],
                            wqk_sb[:, kt, mt * 128:(mt + 1) * 128],
                            xt_sb[:, kt, nt * 512:(nt + 1) * 512],
                            start=False, stop=(kt == KT - 1))
                    nc.vector.tensor_copy(
                        qk_sb[:, mt, nt * 512:(nt + 1) * 512], pmm[:])

                return a, b

            def emit_k256(j):
                # K chunk for tokens j*256:(j+1)*256 (k-tiles 2j, 2j+1);
                # one N=256 GEMM + one CAST, fine-grained for early blocks
                pmm = ps.tile([128, 512], F32, tag="mm")
                for kt in range(KT):
                    nc.tensor.matmul(
                        pmm[:, 0:256],
                        wqk_sb[:, kt, 128:256],
                        xt_sb[:, kt, j * 256:(j + 1) * 256],
                        start=(kt == 0), stop=(kt == KT - 1))
                nc.vector.tensor_copy(
                    qk_sb[:, 1, j * 256:(j + 1) * 256], pmm[:, 0:256])

            def emit_v(t):
                # token-major V for 128-token tile t, BOTH heads:
                # out[tok, vchan] = sum_k xT[k, tok-tile]^T @ wv[k, :]
                pv = ps.tile([128, 512], F32, tag="mm")
                for kt in range(KT):
                    nc.tensor.matmul(
                        pv[:, 0:128],
                        xt_sb[:, kt, t * 128:(t + 1) * 128],
                        wv_sb[:, kt, :],
                        start=(kt == 0), stop=(kt == KT - 1))
                nc.vector.tensor_copy(v_sb[:, t, 0, 0:64], pv[:, 0:64])
                nc.vector.tensor_copy(v_sb[:, t, 1, 64:128], pv[:, 64:128])

            def emit_proj(g, evac=None, dma_eng=None):
                # out_tile = O^T_packed.T @ wproj (K=128, both heads)
                ob = work.tile([128, C], BF, tag="outstage", bufs=4)
                for ntile in range(2):
                    pmm = ps.tile([128, 512], F32, tag="mm")
                    nc.tensor.matmul(
                        pmm[:],
                        otp_sb[:, g * 128:(g + 1) * 128],
                        wproj_sb[:, ntile * 512:(ntile + 1) * 512],
                        start=True, stop=True)
                    if evac == "scalar":
                        nc.scalar.copy(
                            ob[:, ntile * 512:(ntile + 1) * 512], pmm[:])
                    elif evac == "mixed" and ntile == 1:
                        nc.scalar.copy(ob[:, 512:1024], pmm[:])
                    else:
                        nc.vector.tensor_copy(
                            ob[:, ntile * 512:(ntile + 1) * 512], pmm[:])
                (dma_eng or nc.sync).dma_start(
                    out=out[g * 128:(g + 1) * 128, :], in_=ob[:])

            # One attention q-block, BOTH heads.
            def emit_s2_pair(b, qb, extras=None, post=None, carry=None,
                             last=False):
                q0 = b * N + qb * QB
                po0 = ps.tile([128, 512], F32, tag="o0", bufs=1)
                po1 = ps.tile([128, 512], F32, tag="o1", bufs=1)
                from collections import deque as _dq
                pending = _dq()

                def pop_o():
                    es, kt = pending.popleft()
                    vt = b * NK + kt
                    nc.tensor.matmul(
                        po0[:], v_sb[:, vt, 0, :], es[:, 0:512],
                        start=(kt == 0), stop=(kt == NK - 1))
                    nc.tensor.matmul(
                        po1[:], v_sb[:, vt, 1, :], es[:, 512:1024],
                        start=(kt == 0), stop=(kt == NK - 1))

                def normalize(heads=(0, 1), chunks=1):
                    # h0: l at po0 row 64, O at rows 0:64 -> otp[0:64]
                    # h1: l at po1 row 0, O at rows 64:128 -> otp[64:128]
                    cw = QB // chunks
                    for h in heads:
                        po = po0 if h == 0 else po1
                        lrow = work.tile([1, 512], F32, tag="lrow")
                        nc.vector.tensor_copy(
                            lrow[:], po[64:65, :] if h == 0 else po[0:1, :])
                        nc.vector.reciprocal_approx_fast(lrow[:], lrow[:])
                        lb = work.tile([64, 512], F32, tag="lb")
                        nc.gpsimd.partition_broadcast(lb[:], lrow[:])
                        for c in range(chunks):
                            s = c * cw
                            if h == 0:
                                nc.vector.tensor_mul(
                                    otp_sb[0:64, q0 + s:q0 + s + cw],
                                    po[0:64, s:s + cw], lb[:, s:s + cw])
                            else:
                                nc.vector.tensor_mul(
                                    otp_sb[64:128, q0 + s:q0 + s + cw],
                                    po[64:128, s:s + cw], lb[:, s:s + cw])

                def s_exp(kp, j):
                    kt = kp * 2 + j
                    k0 = b * N + kt * 128
                    pst = ps.tile([128, 1024], F32, tag="s")
                    nc.tensor.matmul(
                        pst[:, 0:512],
                        qk_sb[0:64, 1, k0:k0 + 128],
                        qk_sb[0:64, 0, q0:q0 + QB],
                        start=True, stop=True)
                    nc.tensor.matmul(
                        pst[:, 512:1024],
                        qk_sb[64:128, 1, k0:k0 + 128],
                        qk_sb[64:128, 0, q0:q0 + QB],
                        start=True, stop=True)
                    es = work.tile([128, 1024], BF, tag="es", bufs=13)
                    nc.scalar.activation(
                        es[:], pst[:], mybir.ActivationFunctionType.Exp)
                    pending.append((es, kt))

                for kp in range(NK // 2):
                    if carry is not None:
                        carry[0]()
                        carry[0]()
                        if carry[1]():
                            carry = None
                    else:
                        if len(pending) >= 6:
                            pop_o()
                            pop_o()
                    if extras:
                        for u in extras.get(kp, ()):
                            u()
                    s_exp(kp, 0)
                    s_exp(kp, 1)
                    if post:
                        for u in post.get(kp, ()):
                            u()
                if last:
                    # drain head-by-head: h0's normalize chain overlaps h1's
                    # remaining O matmuls on the PE; normalize in 256-col
                    # halves so the first tail projections start early
                    rem = list(pending)
                    pending.clear()
                    for es, kt in rem:
                        nc.tensor.matmul(
                            po0[:], v_sb[:, b * NK + kt, 0, :], es[:, 0:512],
                            start=(kt == 0), stop=(kt == NK - 1))
                    normalize(heads=(0,))
                    for es, kt in rem:
                        nc.tensor.matmul(
                            po1[:], v_sb[:, b * NK + kt, 1, :],
                            es[:, 512:1024],
                            start=(kt == 0), stop=(kt == NK - 1))
                    normalize(heads=(1,))
                    return None

                def carry_pop():
                    if len(pending) > 2:
                        pop_o()

                def carry_fin():
                    if len(pending) > 2:
                        return False
                    rem = list(pending)
                    pending.clear()
                    for es, kt in rem:
                        nc.tensor.matmul(
                            po0[:], v_sb[:, b * NK + kt, 0, :], es[:, 0:512],
                            start=(kt == 0), stop=(kt == NK - 1))
                    normalize(heads=(0,))
                    for es, kt in rem:
                        nc.tensor.matmul(
                            po1[:], v_sb[:, b * NK + kt, 1, :],
                            es[:, 512:1024],
                            start=(kt == 0), stop=(kt == NK - 1))
                    normalize(heads=(1,))
                    return True

                return (carry_pop, carry_fin)

            # -- schedule ---------------------------------------------------
            def U(f, *a):
                return lambda: f(*a)

            # minimal prefix for (b0, qb0): K tokens 0:256 (N=256) and the
            # full Q block 0:512, K/Q interleaved per k-chunk
            pK = ps.tile([128, 512], F32, tag="mm")
            pQ = ps.tile([128, 512], F32, tag="mm")
            for kt in range(KT):
                nc.tensor.matmul(pK[:, 0:256], wqk_sb[:, kt, 128:256],
                                 xt_sb[:, kt, 0:256],
                                 start=(kt == 0), stop=(kt == KT - 1))
                nc.tensor.matmul(pQ[:], wqk_sb[:, kt, 0:128],
                                 xt_sb[:, kt, 0:512],
                                 start=(kt == 0), stop=(kt == KT - 1))
            nc.vector.tensor_copy(qk_sb[:, 1, 0:256], pK[:, 0:256])
            nc.vector.tensor_copy(qk_sb[:, 0, 0:512], pQ[:])

            # (0,0): k256(j) covers b0 K tokens 256j:256j+256 (needed by
            # kp(j-1)); v_t needed by the pop at kp(t//2+3), so emitted by
            # kp(t//2+2); Q(0,1) by next block. kp0/kp1 extras run AFTER the
            # s_exps so the first exps start as early as possible.
            cy = emit_s2_pair(0, 0, post={
                0: [U(emit_k256, 1), U(emit_k256, 2), U(emit_vzero, 0)],
                1: [U(emit_k256, 3), U(emit_k256, 4), U(emit_v, 0)],
            }, extras={
                2: [U(emit_v, 1), U(emit_v, 2), U(emit_v, 3),
                    U(emit_vzero, 1)],
                3: [U(emit_k256, 5), U(emit_v, 4), U(emit_v, 5)],
                4: [U(emit_k256, 6), U(emit_v, 6), U(emit_v, 7),
                    U(emit_vzero, 2)],
                5: [U(emit_k256, 7), U(emit_v, 8), U(emit_v, 9),
                    U(emit_vzero, 3)],
                6: [U(emit_qk, 0, 1), U(emit_v, 10), U(emit_v, 11)],
                7: [U(emit_v, 12), U(emit_v, 13)],
            })
            qk14 = emit_qk_split(1, 4)
            qk02 = emit_qk_split(0, 2)
            qk03 = emit_qk_split(0, 3)
            cy = emit_s2_pair(0, 1, carry=cy, post={
                0: [U(emit_v, 14), U(emit_v, 15)],
            }, extras={
                2: [U(emit_v, 16), U(emit_v, 17)],
                3: [qk02[0]],
                4: [qk02[1], U(emit_v, 18)],
                5: [qk14[0]],
                6: [qk14[1], U(emit_v, 19)],
                7: [qk03[0], U(emit_v, 20)],
            })
            qk15 = emit_qk_split(1, 5)
            qk04 = emit_qk_split(0, 4)
            qk05 = emit_qk_split(0, 5)
            cy = emit_s2_pair(0, 2, carry=cy, extras={
                2: [qk03[1], U(emit_v, 21)],
                3: [qk15[0]],
                4: [qk15[1], U(emit_v, 22)],
                5: [qk04[0]],
                6: [qk04[1], U(emit_v, 23)],
                7: [qk05[0], U(emit_v, 24)],
            })
            qk16 = emit_qk_split(1, 6)
            qk06 = emit_qk_split(0, 6)
            cy = emit_s2_pair(0, 3, carry=cy, extras={
                2: [qk05[1], U(emit_v, 25)],
                3: [qk16[0]],
                4: [qk16[1], U(emit_v, 26)],
                5: [qk06[0]],
                6: [qk06[1], U(emit_proj, 0)],
                7: [U(emit_proj, 1), U(emit_v, 27)],
            })
            qk17 = emit_qk_split(1, 7)
            qk07 = emit_qk_split(0, 7)
            cy = emit_s2_pair(1, 0, carry=cy, extras={
                2: [U(emit_v, 28), U(emit_v, 29)],
                3: [qk17[0]],
                4: [qk17[1], U(emit_v, 30)],
                5: [qk07[0]],
                6: [qk07[1], U(emit_v, 31)],
                7: [U(emit_proj, 2, "mixed"), U(emit_proj, 3, "mixed")],
            })
            cy = emit_s2_pair(1, 1, carry=cy, extras={
                2: [U(emit_proj, 4)],
                3: [U(emit_proj, 5), U(emit_proj, 6)],
                4: [U(emit_proj, 7), U(emit_proj, 8)],
                5: [U(emit_proj, 9), U(emit_proj, 10)],
                6: [U(emit_proj, 11), U(emit_proj, 12)],
                7: [U(emit_proj, 13, "mixed")],
            })
            cy = emit_s2_pair(1, 2, carry=cy, extras={
                2: [U(emit_proj, 14)],
                3: [U(emit_proj, 15), U(emit_proj, 16)],
                4: [U(emit_proj, 17), U(emit_proj, 18)],
                5: [U(emit_proj, 19), U(emit_proj, 20)],
                6: [U(emit_proj, 21), U(emit_proj, 22)],
                7: [U(emit_proj, 23, "mixed")],
            })
            emit_s2_pair(1, 3, carry=cy, last=True, extras={
                2: [U(emit_proj, 24)],
                3: [U(emit_proj, 25)],
                4: [U(emit_proj, 26)],
                5: [U(emit_proj, 27)],
            })
            # tail: ~10 dummy matmuls keep the PE clock-gate open through
            # the final normalize chain; last 4 projections evacuate with one
            # copy on scalar (idle now) and one on vector, out-DMAs spread
            pw2 = ps.tile([128, 512], F32, tag="mm")
            for i in range(10):
                nc.tensor.matmul(pw2[:, 0:128], ident[:], ident[:],
                                 start=True, stop=True)
            emit_proj(28, evac="mixed", dma_eng=nc.scalar)
            emit_proj(29, evac="mixed", dma_eng=nc.sync)
            emit_proj(30, evac="mixed", dma_eng=nc.gpsimd)
            emit_proj(31, evac="mixed", dma_eng=nc.sync)
    nc.compile()
    return nc


def make_in_maps(x, w_qkv, w_proj):
    bf = ml_dtypes.bfloat16
    x2 = x.reshape(T, C)
    xT_np = np.ascontiguousarray(x2.T).astype(bf)
    in_maps = []
    for c in range(NCORES):
        s = c * 128
        wq = w_qkv[:, s:s + 128] * SCALE
        wk = w_qkv[:, C + s:C + s + 128]
        wqk_np = np.ascontiguousarray(
            np.concatenate([wq, wk], axis=1)).astype(bf)
        wv_np = np.ascontiguousarray(
            w_qkv[:, 2 * C + s:2 * C + s + 128]).astype(bf)
        wproj_np = np.ascontiguousarray(w_proj[s:s + 128, :]).astype(bf)
        in_maps.append({"xT": xT_np, "wqk": wqk_np, "wv": wv_np,
                        "wproj": wproj_np})
    return in_maps


def kernel(x, w_qkv, w_proj, b_proj):
    x = np.asarray(x, dtype=np.float32)
    w_qkv = np.asarray(w_qkv, dtype=np.float32)
    w_proj = np.asarray(w_proj, dtype=np.float32)
    b_proj = np.asarray(b_proj, dtype=np.float32)

    if "nc" not in _NC_CACHE:
        _NC_CACHE["nc"] = build()
    nc = _NC_CACHE["nc"]

    in_maps = make_in_maps(x, w_qkv, w_proj)
    res = run_bass_kernel_spmd(nc, in_maps, list(range(NCORES)))
    acc = np.zeros((T, C), dtype=np.float32)
    for r in res.results:
        acc += np.asarray(r["out"], dtype=np.float32)
    acc += b_proj[None, :]
    return acc.reshape(B, N, C)


# revision 22
# speedup vs baseline: 1.0373x; 1.0150x over previous
"""Multi-head attention forward (B=2, N=2048, C=1024, H=16) on 8 TRN2 NeuronCores.

Tensor-parallel over heads: core c owns heads {2c, 2c+1}. Each core computes
QKV projection for its heads, full attention for its 4 (batch, head)
instances, and a partial output projection against its 128 rows of w_proj.
The host sums the 8 partial projections and adds the bias (row-parallel TP;
the all-reduce is the host-side unshard).

Per-core layouts (all matmul inputs bf16, PSUM accumulation f32):
  xT    [1024, 4096]  x^T, channel-major (replicated)
  wqk   [1024, 256]   [Wq_h0|Wq_h1|Wk_h0|Wk_h1] columns, Wq pre-scaled by D^-0.5
  wv    [1024, 128]   [Wv_h0|Wv_h1]
  wproj [128, 1024]   rows 128c:128c+128 of w_proj
  out   [4096, 1024]  bf16 partial projection output

Attention processes BOTH heads per (batch, q-block): the two S^T matmuls
(K=64 contraction each) are issued back-to-back with tile_position row
tiles so they run CONCURRENTLY in disjoint halves of the PE array. Per
k-tile, one [128, 1024] PSUM tile holds [S^T_h0 | S^T_h1] and a single ACT
exp covers both heads.

V is computed directly in token-major layout (stationary = x^T chunks,
moving = wv, N=128) -- no PE transposes. Slot layouts:
  slot0 (h0): [V_h0 (64) | 1 | zeros(63)]   -> po0 = [O_h0 0:64; l_h0 @64]
  slot1 (h1): [1 | zeros(63) | V_h1 (64)]   -> po1 = [l_h1 @0; O_h1 64:128]
so h1's normalized output writes otp partitions 64:128 DIRECTLY (the DVE
mul reads the broadcast tile at partitions 0:64; cross-offset in1 reads are
supported) -- no SBUF repack DMA. l rides the PV matmul via the ones column.

Startup: critical DMAs (wqk + x^T tokens 0:512) spread over all four
queues; ~40 dummy ident matmuls warm the PE HAM clock-gate during the DMA
wait; a short prefix (K tokens 0:256 at N=256, Q tokens 0:512) gets the
first S-pair and exp stream going ~12us earlier than a full-block prefix.

Scheduling: stage-1 and projection PE work is woven into the attention
loops' ACT-wait slots. Exp->O matmuls are skewed 3 kp iterations; each
q-block's last 6 O-accumulations drain 3-per-kp into the next block's
kp0/kp1 with the normalize chain issued at kp1 so the po PSUM banks free
before the block's own O pops begin at kp3.
"""

import numpy as np
import ml_dtypes

import concourse.bass as bass
import concourse.tile as tile
from concourse import bacc, mybir
from concourse.bass_utils import run_bass_kernel_spmd

B, N, C = 2, 2048, 1024
H = 16
D = C // H          # 64
SCALE = D ** -0.5
NCORES = 8
T = B * N           # 4096 tokens
KT = C // 128       # 8 k-tiles over the C contraction
NK = N // 128       # 16 key tiles per sequence
QB = 512            # q block width
BF = mybir.dt.bfloat16
F32 = mybir.dt.float32

_NC_CACHE = {}


def build():
    nc = bacc.Bacc("TRN2", target_bir_lowering=False, debug=False,
                   num_devices=NCORES)
    xT = nc.dram_tensor("xT", [C, T], BF, kind="ExternalInput").ap()
    wqk = nc.dram_tensor("wqk", [C, 256], BF, kind="ExternalInput").ap()
    wv = nc.dram_tensor("wv", [C, 128], BF, kind="ExternalInput").ap()
    wproj = nc.dram_tensor("wproj", [128, C], BF, kind="ExternalInput").ap()
    out = nc.dram_tensor("out", [T, C], BF, kind="ExternalOutput").ap()

    with tile.TileContext(nc) as tc:
        with tc.tile_pool(name="const", bufs=1) as const, \
             tc.tile_pool(name="work", bufs=2) as work, \
             tc.tile_pool(name="ps", bufs=2, space="PSUM") as ps:

            xt_sb = const.tile([128, KT, T], BF, tag="xt")
            wqk_sb = const.tile([128, KT, 256], BF, tag="wqk")
            wv_sb = const.tile([128, KT, 128], BF, tag="wv")
            wproj_sb = const.tile([128, C], BF, tag="wproj")
            qk_sb = const.tile([128, 2, T], BF, tag="qk")   # [qchan|kchan, token]
            # v slots per 128-token tile: [h0: V|1|0pad, h1: 0pad|1|V]
            v_sb = const.tile([128, T // 128, 2, 128], BF, tag="v")
            otp_sb = const.tile([128, T], BF, tag="otp")    # normalized O^T packed
            ident = const.tile([128, 128], BF, tag="ident")

            # ---- critical DMAs: wqk (8 chunks) + x^T tokens 0:512 (8
            # chunks) spread as the FIRST 4 descriptors on each of the four
            # queues. Everything later goes on sync/gpsimd.
            def xt_dma(eng, nt, kt):
                eng.dma_start(
                    out=xt_sb[:, kt, nt * 512:(nt + 1) * 512],
                    in_=xT[kt * 128:(kt + 1) * 128,
                           nt * 512:(nt + 1) * 512])

            for kt in range(KT):
                (nc.sync if kt % 2 == 0 else nc.scalar).dma_start(
                    out=wqk_sb[:, kt, :], in_=wqk[kt * 128:(kt + 1) * 128, :])
                if kt % 2 == 0:
                    xt_dma(nc.gpsimd, 0, kt)
                elif kt % 4 == 1:
                    xt_dma(nc.sync, 0, kt)
                elif kt == 3:
                    xt_dma(nc.scalar, 0, kt)
                else:
                    xt_dma(nc.gpsimd, 0, kt)

            # ---- v-slot constants; then HAM-prewarm dummy matmuls (no DMA
            # deps) so the PE clock-gate opens during the DMA wait
            nc.vector.memset(ident[:], 1.0)
            nc.vector.memset(v_sb[:, :, 0, 64:65], 1.0)
            nc.vector.memset(v_sb[:, :, 1, 0:1], 1.0)
            pwarm = ps.tile([128, 512], F32, tag="mm")
            for i in range(40):
                nc.tensor.matmul(pwarm[:, 0:128], ident[:], ident[:],
                                 start=True, stop=True)

            # ---- remaining input DMAs on sync/gpsimd only
            for kt in range(KT):
                xt_dma(nc.sync if kt % 2 == 0 else nc.gpsimd, 1, kt)
                nc.gpsimd.dma_start(out=wv_sb[:, kt, :],
                                    in_=wv[kt * 128:(kt + 1) * 128, :])
            nc.gpsimd.dma_start(out=wproj_sb[:], in_=wproj[:, :])
            for nt in range(2, T // 512):
                for kt in range(KT):
                    xt_dma(nc.sync if kt % 2 == 0 else nc.gpsimd, nt, kt)

            # -- work units -------------------------------------------------
            def emit_vzero(c):
                # zero-pad of v slots for tiles 8c:8c+8 (pads must be
                # initialized before the first O pop reads those slots)
                nc.vector.memset(v_sb[:, 8 * c:8 * (c + 1), 0, 65:128], 0.0)
                nc.vector.memset(v_sb[:, 8 * c:8 * (c + 1), 1, 1:64], 0.0)

            def emit_qk(mt, nt):
                pmm = ps.tile([128, 512], F32, tag="mm")
                for kt in range(KT):
                    nc.tensor.matmul(
                        pmm[:],
                        wqk_sb[:, kt, mt * 128:(mt + 1) * 128],
                        xt_sb[:, kt, nt * 512:(nt + 1) * 512],
                        start=(kt == 0), stop=(kt == KT - 1))
                nc.vector.tensor_copy(
                    qk_sb[:, mt, nt * 512:(nt + 1) * 512], pmm[:])

            def emit_qk_split(mt, nt):
                # 8-chunk GEMM split into two 4-chunk halves emitted in
                # consecutive extras slots (accumulation group + psum tile
                # stay open across the pair)
                st = {}

                def a():
                    pmm = ps.tile([128, 512], F32, tag="mm", name="pmm")
                    st["p"] = pmm
                    for kt in range(KT // 2):
                        nc.tensor.matmul(
                            pmm[:],
                            wqk_sb[:, kt, mt * 128:(mt + 1) * 128],
                            xt_sb[:, kt, nt * 512:(nt + 1) * 512],
                            start=(kt == 0), stop=False)

                def b():
                    pmm = st["p"]
                    for kt in range(KT // 2, KT):
                        nc.tensor.matmul(
                            pmm[:],
                            wqk_sb[:, kt, mt * 128:(mt + 1) * 128],
                            xt_sb[:, kt, nt * 512:(nt + 1) * 512],
                            start=False, stop=(kt == KT - 1))
                    nc.vector.tensor_copy(
                        qk_sb[:, mt, nt * 512:(nt + 1) * 512], pmm[:])

                return a, b

            def emit_k256(j):
                # K chunk for tokens j*256:(j+1)*256 (k-tiles 2j, 2j+1);
                # one N=256 GEMM + one CAST, fine-grained for early blocks
                pmm = ps.tile([128, 512], F32, tag="mm")
                for kt in range(KT):
                    nc.tensor.matmul(
                        pmm[:, 0:256],
                        wqk_sb[:, kt, 128:256],
                        xt_sb[:, kt, j * 256:(j + 1) * 256],
                        start=(kt == 0), stop=(kt == KT - 1))
                nc.vector.tensor_copy(
                    qk_sb[:, 1, j * 256:(j + 1) * 256], pmm[:, 0:256])

            def emit_v(t):
                # token-major V for 128-token tile t, BOTH heads:
                # out[tok, vchan] = sum_k xT[k, tok-tile]^T @ wv[k, :]
                pv = ps.tile([128, 512], F32, tag="mm")
                for kt in range(KT):
                    nc.tensor.matmul(
                        pv[:, 0:128],
                        xt_sb[:, kt, t * 128:(t + 1) * 128],
                        wv_sb[:, kt, :],
                        start=(kt == 0), stop=(kt == KT - 1))
                nc.vector.tensor_copy(v_sb[:, t, 0, 0:64], pv[:, 0:64])
                nc.vector.tensor_copy(v_sb[:, t, 1, 64:128], pv[:, 64:128])

            def emit_proj(g, evac=None, dma_eng=None):
                # out_tile = O^T_packed.T @ wproj (K=128, both heads)
                ob = work.tile([128, C], BF, tag="outstage", bufs=4)
                for ntile in range(2):
                    pmm = ps.tile([128, 512], F32, tag="mm")
                    nc.tensor.matmul(
                        pmm[:],
                        otp_sb[:, g * 128:(g + 1) * 128],
                        wproj_sb[:, ntile * 512:(ntile + 1) * 512],
                        start=True, stop=True)
                    if evac == "scalar":
                        nc.scalar.copy(
                            ob[:, ntile * 512:(ntile + 1) * 512], pmm[:])
                    elif evac == "mixed" and ntile == 1:
                        nc.scalar.copy(ob[:, 512:1024], pmm[:])
                    else:
                        nc.vector.tensor_copy(
                            ob[:, ntile * 512:(ntile + 1) * 512], pmm[:])
                (dma_eng or nc.sync).dma_start(
                    out=out[g * 128:(g + 1) * 128, :], in_=ob[:])

            # One attention q-block, BOTH heads.
            def emit_s2_pair(b, qb, extras=None, post=None, carry=None,
                             last=False):
                q0 = b * N + qb * QB
                po0 = ps.tile([128, 512], F32, tag="o0", bufs=1)
                po1 = ps.tile([128, 512], F32, tag="o1", bufs=1)
                from collections import deque as _dq
                pending = _dq()

                def pop_o():
                    es, kt = pending.popleft()
                    vt = b * NK + kt
                    nc.tensor.matmul(
                        po0[:], v_sb[:, vt, 0, :], es[:, 0:512],
                        start=(kt == 0), stop=(kt == NK - 1))
                    nc.tensor.matmul(
                        po1[:], v_sb[:, vt, 1, :], es[:, 512:1024],
                        start=(kt == 0), stop=(kt == NK - 1))

                def normalize(heads=(0, 1), chunks=1):
                    # h0: l at po0 row 64, O at rows 0:64 -> otp[0:64]
                    # h1: l at po1 row 0, O at rows 64:128 -> otp[64:128]
                    cw = QB // chunks
                    for h in heads:
                        po = po0 if h == 0 else po1
                        lrow = work.tile([1, 512], F32, tag="lrow")
                        nc.vector.tensor_copy(
                            lrow[:], po[64:65, :] if h == 0 else po[0:1, :])
                        nc.vector.reciprocal_approx_fast(lrow[:], lrow[:])
                        lb = work.tile([64, 512], F32, tag="lb")
                        nc.gpsimd.partition_broadcast(lb[:], lrow[:])
                        for c in range(chunks):
                            s = c * cw
                            if h == 0:
                                nc.vector.tensor_mul(
                                    otp_sb[0:64, q0 + s:q0 + s + cw],
                                    po[0:64, s:s + cw], lb[:, s:s + cw])
                            else:
                                nc.vector.tensor_mul(
                                    otp_sb[64:128, q0 + s:q0 + s + cw],
                                    po[64:128, s:s + cw], lb[:, s:s + cw])

                def s_exp(kp, j):
                    kt = kp * 2 + j
                    k0 = b * N + kt * 128
                    pst = ps.tile([128, 1024], F32, tag="s")
                    nc.tensor.matmul(
                        pst[:, 0:512],
                        qk_sb[0:64, 1, k0:k0 + 128],
                        qk_sb[0:64, 0, q0:q0 + QB],
                        start=True, stop=True)
                    nc.tensor.matmul(
                        pst[:, 512:1024],
                        qk_sb[64:128, 1, k0:k0 + 128],
                        qk_sb[64:128, 0, q0:q0 + QB],
                        start=True, stop=True)
                    es = work.tile([128, 1024], BF, tag="es", bufs=13)
                    nc.scalar.activation(
                        es[:], pst[:], mybir.ActivationFunctionType.Exp)
                    pending.append((es, kt))

                for kp in range(NK // 2):
                    if carry is not None:
                        carry[0]()
                        carry[0]()
                        if carry[1]():
                            carry = None
                    else:
                        if len(pending) >= 6:
                            pop_o()
                            pop_o()
                    if extras:
                        for u in extras.get(kp, ()):
                            u()
                    s_exp(kp, 0)
                    s_exp(kp, 1)
                    if post:
                        for u in post.get(kp, ()):
                            u()
                if last:
                    # drain head-by-head: h0's normalize chain overlaps h1's
                    # remaining O matmuls on the PE; normalize in 256-col
                    # halves so the first tail projections start early
                    rem = list(pending)
                    pending.clear()
                    for es, kt in rem:
                        nc.tensor.matmul(
                            po0[:], v_sb[:, b * NK + kt, 0, :], es[:, 0:512],
                            start=(kt == 0), stop=(kt == NK - 1))
                    normalize(heads=(0,))
                    for es, kt in rem:
                        nc.tensor.matmul(
                            po1[:], v_sb[:, b * NK + kt, 1, :],
                            es[:, 512:1024],
                            start=(kt == 0), stop=(kt == NK - 1))
                    normalize(heads=(1,))
                    return None

                def carry_pop():
                    if len(pending) > 2:
                        pop_o()

                def carry_fin():
                    if len(pending) > 2:
                        return False
                    rem = list(pending)
                    pending.clear()
                    for es, kt in rem:
                        nc.tensor.matmul(
                            po0[:], v_sb[:, b * NK + kt, 0, :], es[:, 0:512],
                            start=(kt == 0), stop=(kt == NK - 1))
                    normalize(heads=(0,))
                    for es, kt in rem:
                        nc.tensor.matmul(
                            po1[:], v_sb[:, b * NK + kt, 1, :],
                            es[:, 512:1024],
                            start=(kt == 0), stop=(kt == NK - 1))
                    normalize(heads=(1,))
                    return True

                return (carry_pop, carry_fin)

            # -- schedule ---------------------------------------------------
            def U(f, *a):
                return lambda: f(*a)

            # minimal prefix for (b0, qb0): K tokens 0:256 (N=256) and the
            # full Q block 0:512, K/Q interleaved per k-chunk
            pK = ps.tile([128, 512], F32, tag="mm")
            pQ = ps.tile([128, 512], F32, tag="mm")
            for kt in range(KT):
                nc.tensor.matmul(pK[:, 0:256], wqk_sb[:, kt, 128:256],
                                 xt_sb[:, kt, 0:256],
                                 start=(kt == 0), stop=(kt == KT - 1))
                nc.tensor.matmul(pQ[:], wqk_sb[:, kt, 0:128],
                                 xt_sb[:, kt, 0:512],
                                 start=(kt == 0), stop=(kt == KT - 1))
            nc.vector.tensor_copy(qk_sb[:, 1, 0:256], pK[:, 0:256])
            nc.vector.tensor_copy(qk_sb[:, 0, 0:512], pQ[:])

            # (0,0): k256(j) covers b0 K tokens 256j:256j+256 (needed by
            # kp(j-1)); v_t needed by the pop at kp(t//2+3), so emitted by
            # kp(t//2+2); Q(0,1) by next block. kp0/kp1 extras run AFTER the
            # s_exps so the first exps start as early as possible.
            cy = emit_s2_pair(0, 0, post={
                0: [U(emit_k256, 1), U(emit_k256, 2), U(emit_vzero, 0)],
                1: [U(emit_k256, 3), U(emit_k256, 4), U(emit_v, 0)],
            }, extras={
                2: [U(emit_v, 1), U(emit_v, 2), U(emit_v, 3),
                    U(emit_vzero, 1)],
                3: [U(emit_k256, 5), U(emit_v, 4), U(emit_v, 5)],
                4: [U(emit_k256, 6), U(emit_v, 6), U(emit_v, 7),
                    U(emit_vzero, 2)],
                5: [U(emit_k256, 7), U(emit_v, 8), U(emit_v, 9),
                    U(emit_vzero, 3)],
                6: [U(emit_qk, 0, 1), U(emit_v, 10), U(emit_v, 11)],
                7: [U(emit_v, 12), U(emit_v, 13)],
            })
            qk14 = emit_qk_split(1, 4)
            qk02 = emit_qk_split(0, 2)
            qk03 = emit_qk_split(0, 3)
            cy = emit_s2_pair(0, 1, carry=cy, post={
                0: [U(emit_v, 14), U(emit_v, 15)],
            }, extras={
                2: [U(emit_v, 16), U(emit_v, 17)],
                3: [qk02[0]],
                4: [qk02[1], U(emit_v, 18)],
                5: [qk14[0]],
                6: [qk14[1], U(emit_v, 19)],
                7: [qk03[0], U(emit_v, 20)],
            })
            qk15 = emit_qk_split(1, 5)
            qk04 = emit_qk_split(0, 4)
            qk05 = emit_qk_split(0, 5)
            cy = emit_s2_pair(0, 2, carry=cy, extras={
                2: [qk03[1], U(emit_v, 21)],
                3: [qk15[0]],
                4: [qk15[1], U(emit_v, 22)],
                5: [qk04[0]],
                6: [qk04[1], U(emit_v, 23)],
                7: [qk05[0], U(emit_v, 24)],
            })
            qk16 = emit_qk_split(1, 6)
            qk06 = emit_qk_split(0, 6)
            cy = emit_s2_pair(0, 3, carry=cy, extras={
                2: [qk05[1], U(emit_v, 25)],
                3: [qk16[0]],
                4: [qk16[1], U(emit_v, 26)],
                5: [qk06[0]],
                6: [qk06[1], U(emit_proj, 0)],
                7: [U(emit_proj, 1), U(emit_v, 27)],
            })
            qk17 = emit_qk_split(1, 7)
            qk07 = emit_qk_split(0, 7)
            cy = emit_s2_pair(1, 0, carry=cy, extras={
                2: [U(emit_v, 28), U(emit_v, 29)],
                3: [qk17[0]],
                4: [qk17[1], U(emit_v, 30)],
                5: [qk07[0]],
                6: [qk07[1], U(emit_v, 31)],
                7: [U(emit_proj, 2, "mixed"), U(emit_proj, 3, "mixed")],
            })
            cy = emit_s2_pair(1, 1, carry=cy, extras={
                2: [U(emit_proj, 4)],
                3: [U(emit_proj, 5), U(emit_proj, 6)],
                4: [U(emit_proj, 7), U(emit_proj, 8)],
                5: [U(emit_proj, 9), U(emit_proj, 10)],
                6: [U(emit_proj, 11), U(emit_proj, 12)],
                7: [U(emit_proj, 13, "mixed")],
            })
            cy = emit_s2_pair(1, 2, carry=cy, extras={
                2: [U(emit_proj, 14)],
                3: [U(emit_proj, 15), U(emit_proj, 16)],
                4: [U(emit_proj, 17), U(emit_proj, 18)],
                5: [U(emit_proj, 19), U(emit_proj, 20)],
                6: [U(emit_proj, 21), U(emit_proj, 22)],
                7: [U(emit_proj, 23, "mixed")],
            })
            emit_s2_pair(1, 3, carry=cy, last=True, extras={
                2: [U(emit_proj, 24)],
                3: [U(emit_proj, 25)],
                4: [U(emit_proj, 26)],
                5: [U(emit_proj, 27)],
            })
            # tail: ~10 dummy matmuls keep the PE clock-gate open through
            # the final normalize chain; last 4 projections evacuate with one
            # copy on scalar (idle now) and one on vector, out-DMAs spread
            pw2 = ps.tile([128, 512], F32, tag="mm")
            for i in range(10):
                nc.tensor.matmul(pw2[:, 0:128], ident[:], ident[:],
                                 start=True, stop=True)
            emit_proj(28, evac="mixed", dma_eng=nc.scalar)
            emit_proj(29, evac="mixed", dma_eng=nc.sync)
            emit_proj(30, evac="mixed", dma_eng=nc.gpsimd)
            emit_proj(31, evac="mixed", dma_eng=nc.sync)
    nc.compile()
    return nc


def make_in_maps(x, w_qkv, w_proj):
    bf = ml_dtypes.bfloat16
    x2 = x.reshape(T, C)
    xT_np = np.ascontiguousarray(x2.T).astype(bf)
    in_maps = []
    for c in range(NCORES):
        s = c * 128
        wq = w_qkv[:, s:s + 128] * SCALE
        wk = w_qkv[:, C + s:C + s + 128]
        wqk_np = np.ascontiguousarray(
            np.concatenate([wq, wk], axis=1)).astype(bf)
        wv_np = np.ascontiguousarray(
            w_qkv[:, 2 * C + s:2 * C + s + 128]).astype(bf)
        wproj_np = np.ascontiguousarray(w_proj[s:s + 128, :]).astype(bf)
        in_maps.append({"xT": xT_np, "wqk": wqk_np, "wv": wv_np,
                        "wproj": wproj_np})
    return in_maps


def kernel(x, w_qkv, w_proj, b_proj):
    x = np.asarray(x, dtype=np.float32)
    w_qkv = np.asarray(w_qkv, dtype=np.float32)
    w_proj = np.asarray(w_proj, dtype=np.float32)
    b_proj = np.asarray(b_proj, dtype=np.float32)

    if "nc" not in _NC_CACHE:
        _NC_CACHE["nc"] = build()
    nc = _NC_CACHE["nc"]

    in_maps = make_in_maps(x, w_qkv, w_proj)
    res = run_bass_kernel_spmd(nc, in_maps, list(range(NCORES)))
    acc = np.zeros((T, C), dtype=np.float32)
    for r in res.results:
        acc += np.asarray(r["out"], dtype=np.float32)
    acc += b_proj[None, :]
    return acc.reshape(B, N, C)
